# revision 48
# baseline (speedup 1.0000x reference)
"""Trainium2 Bass kernel for nn_Attention_5093831213465.

Reference computation (per sample, x_b: [256, 4096]):
  q = Wq @ x_b                       [32, 4096]
  k = maxpool2(Wk @ x_b)             [32, 1024]
  v = maxpool2(Wv @ x_b)             [128, 1024]
  attn = softmax_over_k(k^T @ q)     [1024, 4096]
  out  = Wa @ (v @ attn)             [256, 4096]
  y    = gamma * out + x_b

Sharding: data-parallel over batch, 2 samples per core on 8 cores.

Design (tuned against the TimelineSim cost model: a matmul costs
output_free_cols * pe_cycle * cycles_per_row regardless of contraction
or partition dims; fp8 DoubleRow halves cycles/row; PE drops out of its
fast p-state after any idle gap, so the PE queue must never drain):
- x ships three ways: bf16 (q/k conv fmap + residual), fp8e4 (v conv),
  never f32 -- halves the x DMA traffic that paces the warmup.
- q/k conv: stacked [Wq;Wk] bf16 weight, out [64, 512] per chunk.
- v conv: fp8e4 DoubleRow, both 128-row contraction tiles in one pass
  (0.5 cyc/row).  fp8 noise only touches the value path, never the
  softmax logits (q/k stay bf16xbf16 -> fp32 PSUM).
- softmax denominator: each exp group's two kt tiles are pair-summed
  in bf16 (SBUF->SBUF, split between DVE and the otherwise-idle gpsimd
  engine), then ones-MATRIX [128,128] stationary matmuls accumulate the
  pairs -- half the PE columns, and the PSUM result lands replicated
  across all 128 partitions -> no partition_broadcast; reciprocal runs
  full width and feeds the normalize multiply directly.
- E = exp(logits) in bf16; vT in bf16 via bf16 PE transpose (1 cyc/row).
- Software pipeline: attention+exp run THREE chunks ahead of the U/
  denominator matmuls (PE never waits on the Activation engine), Wa lags
  one more chunk.  All 8 denominator matmuls issue at the top of each
  iteration so the reciprocal->normalize DVE chain overlaps the U
  matmuls instead of gating the next iteration.
- Sample-0 conv phase is x-DMA-paced; chunks 0-2 attention groups and
  chunks 0-1's U/denominator accumulation interleave into it as filler.
  Sample-1 convs spread over sample-0's attention iterations 1-5.
- engine split: Act = exp + q-copies + steady-state vT copies; DVE =
  pools, reciprocal, normalize, y-adds, conv-phase vT copies (gpsimd
  cannot touch PSUM); x loads on the SP HWDGE queue, y stores on Act's.
- y is stored bf16 (host upcasts to f32): halves store traffic and the
  tail drain; adds ~1e-3 rounding, still 2.7x under the 2e-2 gate.
- PSUM banks: attn ring 4 + conv/Wa-out shared ring 2 + denom 1 + U 1.
"""

import sys

import numpy as np

if "/opt/trn_rl_repo" not in sys.path:
    sys.path.insert(0, "/opt/trn_rl_repo")

B, C, H, W = 16, 256, 64, 64
CA = C // 8          # 32  attn channels
CS = C // 2          # 128 value channels
HWF = H * W          # 4096 spatial positions
HWP = HWF // 4       # 1024 pooled positions
SPC = 2              # samples per core
NCORES = 8
CHUNK = 512          # qq columns per chunk
NCHUNK = HWF // CHUNK       # 8
KT = HWP // 128             # 8 kk tiles of 128
KG = 2                      # kk tiles per exp group
NG = KT // KG               # 4 groups

_built = {}
LABELS = {}


def _lab(inst, txt):
    try:
        LABELS[inst.ins.name] = txt
    except Exception:
        pass


def _build_program():
    from contextlib import ExitStack

    import concourse.bass as bass
    import concourse.tile as tile
    from concourse import bacc, mybir

    f32 = mybir.dt.float32
    f32r = mybir.dt.float32r
    bf16 = mybir.dt.bfloat16
    fp8 = mybir.dt.float8e4
    Exp = mybir.ActivationFunctionType.Exp
    DR = mybir.MatmulPerfMode.DoubleRow

    nc = bacc.Bacc(
        "TRN2", target_bir_lowering=False, debug=False, enable_asserts=False
    )

    x_d = nc.dram_tensor("x", [SPC, C, HWF], bf16, kind="ExternalInput").ap()
    x8_d = nc.dram_tensor("x8", [SPC, C, HWF], fp8, kind="ExternalInput").ap()
    wqk_d = nc.dram_tensor("wqkT", [C, 2 * CA], bf16, kind="ExternalInput").ap()
    wv8_d = nc.dram_tensor("wv8T", [C, CS], fp8, kind="ExternalInput").ap()
    wa_d = nc.dram_tensor("waTg", [CS, C], f32r, kind="ExternalInput").ap()
    idb_d = nc.dram_tensor("identb", [128, 128], bf16, kind="ExternalInput").ap()
    on_d = nc.dram_tensor("onesm", [128, 128], bf16, kind="ExternalInput").ap()
    y_d = nc.dram_tensor("y", [SPC, C, HWF], bf16, kind="ExternalOutput").ap()

    with tile.TileContext(nc) as tc, ExitStack() as ctx:
        consts = ctx.enter_context(tc.tile_pool(name="consts", bufs=1))
        xp = ctx.enter_context(tc.tile_pool(name="xp", bufs=1))
        x8p = ctx.enter_context(tc.tile_pool(name="x8p", bufs=1))
        qp = ctx.enter_context(tc.tile_pool(name="qp", bufs=1))
        kvp = ctx.enter_context(tc.tile_pool(name="kvp", bufs=1))
        ep = ctx.enter_context(tc.tile_pool(name="ep", bufs=14))
        e2p = ctx.enter_context(tc.tile_pool(name="e2p", bufs=4))
        sp = ctx.enter_context(tc.tile_pool(name="sp", bufs=3))
        yp = ctx.enter_context(tc.tile_pool(name="yp", bufs=4))
        psA = ctx.enter_context(tc.tile_pool(name="psA", bufs=2, space="PSUM"))
        psCv = ctx.enter_context(tc.tile_pool(name="psCv", bufs=2, space="PSUM"))
        psS = ctx.enter_context(tc.tile_pool(name="psS", bufs=1, space="PSUM"))
        psUu = ctx.enter_context(tc.tile_pool(name="psUu", bufs=1, space="PSUM"))

        # constants + x loads on the SP queue, ordered so conv(0,0) can
        # start as early as possible: conv weights, first 512-col slices
        # of x/x8, then everything else.
        xr = {}
        x8r = {}
        for s in range(SPC):
            xr[s] = xp.tile([128, 2, HWF], bf16, name=f"xr{s}", tag=f"xr{s}")
            x8r[s] = x8p.tile([128, 2, HWF], fp8, name=f"x8r{s}", tag=f"x8{s}")
        xv = {s: x_d[s].rearrange("(t p) m -> p t m", p=128) for s in range(SPC)}
        x8v = {s: x8_d[s].rearrange("(t p) m -> p t m", p=128) for s in range(SPC)}

        def xload(s, lo, hi, with8, eng=None):
            eng = eng or nc.sync
            cs = slice(lo, hi)
            eng.dma_start(xr[s][:, :, cs], xv[s][:, :, cs])
            if with8:
                eng.dma_start(x8r[s][:, :, cs], x8v[s][:, :, cs])

        cs0 = slice(0, 512)
        nc.sync.dma_start(xr[0][:, :, cs0], xv[0][:, :, cs0])
        wqk = consts.tile([128, 2, 2 * CA], bf16, name="wqk")
        nc.scalar.dma_start(wqk[:], wqk_d.rearrange("(t p) m -> p t m", p=128))
        nc.scalar.dma_start(x8r[0][:, :, cs0], x8v[0][:, :, cs0])
        wv8 = consts.tile([128, 2, CS], fp8, name="wv8")
        nc.scalar.dma_start(wv8[:], wv8_d.rearrange("(t p) m -> p t m", p=128))

        wa = consts.tile([128, 2, 128], f32r, name="wa")
        nc.sync.dma_start(wa[:], wa_d.rearrange("p (t m) -> p t m", t=2))
        identb = consts.tile([128, 128], bf16, name="identb")
        nc.sync.dma_start(identb[:], idb_d)
        onesm = consts.tile([128, 128], bf16, name="onesm")
        nc.sync.dma_start(onesm[:], on_d)
        # prime the Exp activation table so the 1.3us table load is off
        # the critical path
        prime = consts.tile([1, 2], f32, name="prime")
        nc.scalar.activation(prime[:], onesm[0:1, 0:2], Exp)

        xload(0, 512, 1024, True)
        xload(0, 1024, 2048, True)
        xload(0, 2048, 3072, True)
        xload(0, 3072, 4096, True)
        for h in range(4):
            xload(1, h * 1024, (h + 1) * 1024, h % 2 == 0)
            if h % 2 == 0:
                cs = slice(h * 1024 + 1024, h * 1024 + 2048)
                nc.sync.dma_start(x8r[1][:, :, cs], x8v[1][:, :, cs])

        # per-sample conv products
        qs = {}     # (s, ck) -> [32, 512] f32r q chunk
        kph = {}    # (s, kt) -> [32, 128] f32r pooled k tile
        vTh = {}    # (s, kt) -> [128, 128] bf16 v^T tile

        def conv_chunk(s, ck):
            """1x1 convs + pools + v transpose for chunk ck of sample s."""
            cs = slice(ck * CHUNK, (ck + 1) * CHUNK)
            # q/k conv, f32r, out [64, 512]
            pqk = psCv.tile([64, CHUNK], f32, name=f"pqk{s}{ck}", tag="cv")
            for t in range(2):
                _lab(nc.tensor.matmul(
                    pqk[:],
                    wqk[:, t, :],
                    xr[s][:, t, cs],
                    start=(t == 0),
                    stop=(t == 1),
                ), f"qkconv s{s}c{ck}t{t}")
            # v conv, fp8 DoubleRow: both 128-row tiles in one pass
            pv = psCv.tile([128, CHUNK], f32, name=f"pv{s}{ck}", tag="cv")
            _lab(nc.tensor.matmul(
                pv[:],
                wv8[:],
                x8r[s][:, :, cs],
                start=True,
                stop=True,
                perf_mode=DR,
            ), f"vconv s{s}c{ck}")
            # q chunk to SBUF on Act (keeps DVE off the critical chains)
            q = qp.tile([CA, CHUNK], f32r, name=f"q{s}{ck}", tag=f"q{s}{ck}")
            nc.scalar.copy(q[:], pqk[0:CA, :])
            qs[(s, ck)] = q
            # pooled k tile (DVE), cross-partition write 32:64 -> 0:32
            kt_ = kvp.tile([CA, 128], f32r, name=f"kp{s}{ck}", tag=f"kp{s}{ck}")
            nc.vector.tensor_reduce(
                kt_[:].rearrange("p (h2 w2) -> p h2 w2", h2=4),
                pqk[CA : 2 * CA, :].rearrange(
                    "p (h2 dh w2 dw) -> p h2 w2 dh dw", h2=4, dh=2, w2=32, dw=2
                ),
                axis=mybir.AxisListType.XY,
                op=mybir.AluOpType.max,
            )
            kph[(s, ck)] = kt_
            # pooled v tile (DVE) in bf16
            vph = kvp.tile([128, 128], bf16, name=f"vp{s}{ck}", tag=f"vp{s}{ck}")
            nc.vector.tensor_reduce(
                vph[:].rearrange("p (h2 w2) -> p h2 w2", h2=4),
                pv[:].rearrange(
                    "p (h2 dh w2 dw) -> p h2 w2 dh dw", h2=4, dh=2, w2=32, dw=2
                ),
                axis=mybir.AxisListType.XY,
                op=mybir.AluOpType.max,
            )
            vphs[(s, ck)] = vph

        vphs = {}

        def transpose_chunk(s, ck, dve_copy=False):
            # v^T via bf16 PE transpose, copy out on Act (or DVE near the
            # conv-phase tail where Act's queue is full of exps)
            ptr = psCv.tile([128, 128], bf16, name=f"pt{s}{ck}", tag="cv")
            _lab(nc.tensor.transpose(ptr[:], vphs[(s, ck)][:], identb[:]), f"transp s{s}c{ck}")
            vT = kvp.tile([128, 128], bf16, name=f"vT{s}{ck}", tag=f"vT{s}{ck}")
            if dve_copy:
                nc.vector.tensor_copy(vT[:], ptr[:])
            else:
                nc.scalar.copy(vT[:], ptr[:])
            vTh[(s, ck)] = vT

        # pipelined attention state
        st = {}

        def attn_groups(s, ck, glist):
            d = st[(s, ck)]
            for g in glist:
                pa = psA.tile([128, KG, CHUNK], f32, name=f"pa{s}{ck}{g}", tag="attn")
                for j in range(KG):
                    kt = g * KG + j
                    _lab(nc.tensor.matmul(
                        pa[:, j, :],
                        kph[(s, kt)][:],
                        qs[(s, ck)][:],
                        start=True,
                        stop=True,
                    ), f"attn s{s}c{ck}g{g}j{j}")
                eg = ep.tile([128, KG, CHUNK], bf16, name=f"E{s}{ck}{g}", tag="E")
                nc.scalar.activation(eg[:], pa[:], Exp)
                d["E"][g] = eg

        def u_mms(s, ck, gs):
            """U matmuls for exp groups gs of chunk ck."""
            d = st[(s, ck)]
            for g in gs:
                for j in range(KG):
                    kt = g * KG + j
                    _lab(nc.tensor.matmul(
                        d["psU"][:],
                        vTh[(s, kt)][:],
                        d["E"][g][:, j, :],
                        start=(kt == 0),
                        stop=(kt == KT - 1),
                    ), f"U s{s}c{ck}g{g}j{j}")

        def build_pairs(s, ck):
            """bf16 pair-sums of each exp group's two kt tiles, halving
            the denominator matmul count on the PE.  Pure SBUF->SBUF, so
            half run on the otherwise-idle gpsimd engine."""
            d = st[(s, ck)]
            pair = e2p.tile([128, NG, CHUNK], bf16, name=f"e2{s}{ck}", tag="e2")
            for g in range(NG):
                eng = nc.vector if g % 2 == 0 else nc.gpsimd
                eng.tensor_tensor(
                    pair[:, g, :],
                    d["E"][g][:, 0, :],
                    d["E"][g][:, 1, :],
                    op=mybir.AluOpType.add,
                )
            d["pair"] = pair

        def s_mms_paired(s, ck):
            d = st[(s, ck)]
            for g in range(NG):
                _lab(nc.tensor.matmul(
                    d["psS"][:],
                    onesm[:],
                    d["pair"][:, g, :],
                    start=(g == 0),
                    stop=(g == NG - 1),
                ), f"S2 s{s}c{ck}g{g}")

        def s_mms(s, ck, gs=tuple(range(NG))):
            """denominator matmuls of chunk ck (inputs ready: attn runs
            two chunks ahead), so the reciprocal can fire mid-iteration."""
            d = st[(s, ck)]
            for g in gs:
                for j in range(KG):
                    kt = g * KG + j
                    _lab(nc.tensor.matmul(
                        d["psS"][:],
                        onesm[:],
                        d["E"][g][:, j, :],
                        start=(kt == 0),
                        stop=(kt == KT - 1),
                    ), f"S s{s}c{ck}g{g}j{j}")

        def recip_chunk(s, ck):
            d = st[(s, ck)]
            rb = sp.tile([128, CHUNK], f32, name=f"rb{s}{ck}", tag="rb")
            nc.vector.reciprocal_approx_fast(rb[:], d["psS"][:])
            d["rb"] = rb

        def un_chunk(s, ck):
            d = st[(s, ck)]
            un = sp.tile([128, CHUNK], f32r, name=f"un{s}{ck}", tag="un")
            nc.vector.tensor_mul(un[:], d["psU"][:], d["rb"][:])
            d["un"] = un

        def wa_chunk(s, ck, split_store=False):
            d = st[(s, ck)]
            cs = slice(ck * CHUNK, (ck + 1) * CHUNK)
            yt = yp.tile([128, 2, CHUNK], bf16, name=f"yt{s}{ck}", tag="yt")
            yv = y_d[s].rearrange("(t p) m -> p t m", p=128)
            pos = []
            for mt in range(2):
                po = psCv.tile([128, CHUNK], f32, name=f"po{s}{ck}{mt}", tag="cv")
                _lab(nc.tensor.matmul(
                    po[:], wa[:, mt, :], d["un"][:], start=True, stop=True
                ), f"wa s{s}c{ck}mt{mt}")
                pos.append(po)
            for mt in range(2):
                nc.vector.tensor_tensor(
                    yt[:, mt, :],
                    pos[mt][:],
                    xr[s][:, mt, cs],
                    op=mybir.AluOpType.add,
                )
                if split_store:
                    nc.scalar.dma_start(yv[:, mt, cs], yt[:, mt, :])
            if not split_store:
                nc.scalar.dma_start(yv[:, :, cs], yt[:])

        def new_chunk_state(s, ck):
            st[(s, ck)] = {
                "E": [None] * NG,
                "psS": psS.tile([128, CHUNK], f32, name=f"ps{s}{ck}", tag="s"),
                "psU": psUu.tile([128, CHUNK], f32, name=f"pu{s}{ck}", tag="u"),
            }

        # ---- conv phase for sample 0.  x arrives over ~13us, so the
        # conv chain is DMA-paced; attention groups of chunks 0 and 1 are
        # issued as soon as their k tiles exist to keep the PE fed.
        # Transposes run two chunks behind their v-pool.
        new_chunk_state(0, 0)
        new_chunk_state(0, 1)
        new_chunk_state(0, 2)
        conv_filler = {
            3: [(0, 0)], 4: [(0, 1)], 5: [(1, 0), (0, 2)],
            6: [(1, 1), (1, 2), (2, 0)], 7: [(0, 3), (1, 3), (2, 1)],
        }
        for ck in range(NCHUNK):
            conv_chunk(0, ck)
            if ck >= 2:
                transpose_chunk(0, ck - 2, dve_copy=True)
            for c, g in conv_filler.get(ck, []):
                attn_groups(0, c, [g])
            # chunk (0,0)'s U/denominator accumulation starts as soon as
            # its exp groups and vT tiles exist
            if ck >= 5:
                s_mms(0, 0, [ck - 5])
                u_mms(0, 0, [ck - 5])
        transpose_chunk(0, NCHUNK - 2, dve_copy=True)
        transpose_chunk(0, NCHUNK - 1, dve_copy=True)
        attn_groups(0, 2, [2, 3])
        s_mms(0, 0, [3])
        u_mms(0, 0, [3])
        recip_chunk(0, 0)
        un_chunk(0, 0)
        # chunk (0,1)'s accumulation also starts here: its exps and vT
        # tiles all exist by the end of the conv phase
        s_mms(0, 1, [0, 1])
        u_mms(0, 1, [0, 1])

        # ---- pipelined attention over 16 virtual chunks: attention runs
        # TWO chunks ahead of the U/denominator matmuls (so exp latency
        # never gates the PE), Wa lags U by one chunk.
        VC = [(s, ck) for s in range(SPC) for ck in range(NCHUNK)]

        for i, (s, ck) in enumerate(VC):
            ahead = VC[i + 3] if i + 3 < len(VC) else None
            prev = VC[i - 1] if i >= 1 else None
            if ahead is not None and i >= 0:
                if ahead not in st:
                    new_chunk_state(*ahead)
                    attn_groups(*ahead, [0, 1])
            last = i == len(VC) - 1
            if i == 1:
                s_mms(s, ck, [2, 3])
                recip_chunk(s, ck)
            elif i > 1:
                s_mms_paired(s, ck)
                recip_chunk(s, ck)
            if i + 1 < len(VC) and i >= 1:
                build_pairs(*VC[i + 1])
            if ahead is not None and ahead[0] * NCHUNK + ahead[1] > 2:
                attn_groups(*ahead, [2])
            if i > 1:
                u_mms(s, ck, [0, 1])
            if i > 0 and last:
                u_mms(s, ck, [2, 3])
                un_chunk(s, ck)
            if prev is not None:
                wa_chunk(*prev, split_store=last)
            # sample-1 convs spread over iters 1-5 (pairs then singles),
            # issued before the ahead-attn g3 so iter 5's chunk (1,0)
            # lookahead has every k tile
            s1c = {1: [0, 1], 2: [2, 3], 3: [4, 5], 4: [6], 5: [7]}.get(i, [])
            for c1 in s1c:
                conv_chunk(1, c1)
                if c1 >= 1:
                    transpose_chunk(1, c1 - 1)
            if ahead is not None and ahead[0] * NCHUNK + ahead[1] > 2:
                attn_groups(*ahead, [3])
            if i > 0 and not last:
                u_mms(s, ck, [2, 3])
                un_chunk(s, ck)
            if i == 6:
                transpose_chunk(1, NCHUNK - 1, dve_copy=True)

        # drain tail
        wa_chunk(*VC[-1], split_store=True)

    nc.compile()
    return nc


def _get_program():
    if "nc" not in _built:
        _built["nc"] = _build_program()
    return _built["nc"]


def _make_in_maps(x, Wq, Wk, Wv, Wa, gamma):
    import ml_dtypes

    fp8 = ml_dtypes.float8_e4m3
    xf = np.ascontiguousarray(np.asarray(x, dtype=np.float32).reshape(B, C, HWF))
    x8 = np.ascontiguousarray(xf.astype(fp8))
    x = np.ascontiguousarray(xf.astype(ml_dtypes.bfloat16))
    wqkT = np.ascontiguousarray(
        np.concatenate([np.asarray(Wq), np.asarray(Wk)], axis=0).T.astype(
            ml_dtypes.bfloat16
        )
    )
    wv8T = np.ascontiguousarray(np.asarray(Wv).T.astype(np.float32)).astype(fp8)
    waTg = np.ascontiguousarray(
        (float(np.asarray(gamma).reshape(-1)[0]) * np.asarray(Wa)).T.astype(np.float32)
    )
    identb = np.eye(128, dtype=np.float32).astype(ml_dtypes.bfloat16)
    onesm = np.ones((128, 128), dtype=np.float32).astype(ml_dtypes.bfloat16)
    return [
        {
            "x": np.ascontiguousarray(x[c * SPC : (c + 1) * SPC]),
            "x8": np.ascontiguousarray(x8[c * SPC : (c + 1) * SPC]),
            "wqkT": wqkT,
            "wv8T": wv8T,
            "waTg": waTg,
            "identb": identb,
            "onesm": onesm,
        }
        for c in range(NCORES)
    ]


def kernel(x, Wq, Wk, Wv, Wa, gamma):
    from concourse import bass_utils

    nc = _get_program()
    in_maps = _make_in_maps(x, Wq, Wk, Wv, Wa, gamma)
    res = bass_utils.run_bass_kernel_spmd(
        nc, in_maps, core_ids=list(range(NCORES))
    )
    out = np.concatenate(
        [np.asarray(res.results[c]["y"], dtype=np.float32) for c in range(NCORES)],
        axis=0,
    )
    return out.reshape(B, C, H, W)


# revision 49
# speedup vs baseline: 1.0019x; 1.0019x over previous
"""Trainium2 Bass kernel for nn_Attention_5093831213465.

Reference computation (per sample, x_b: [256, 4096]):
  q = Wq @ x_b                       [32, 4096]
  k = maxpool2(Wk @ x_b)             [32, 1024]
  v = maxpool2(Wv @ x_b)             [128, 1024]
  attn = softmax_over_k(k^T @ q)     [1024, 4096]
  out  = Wa @ (v @ attn)             [256, 4096]
  y    = gamma * out + x_b

Sharding: data-parallel over batch, 2 samples per core on 8 cores.

Design (tuned against the TimelineSim cost model: a matmul costs
output_free_cols * pe_cycle * cycles_per_row regardless of contraction
or partition dims; fp8 DoubleRow halves cycles/row; PE drops out of its
fast p-state after any idle gap, so the PE queue must never drain):
- x ships three ways: bf16 (q/k conv fmap + residual), fp8e4 (v conv),
  never f32 -- halves the x DMA traffic that paces the warmup.
- q/k conv: stacked [Wq;Wk] bf16 weight, out [64, 512] per chunk.
- v conv: fp8e4 DoubleRow, both 128-row contraction tiles in one pass
  (0.5 cyc/row).  fp8 noise only touches the value path, never the
  softmax logits (q/k stay bf16xbf16 -> fp32 PSUM).
- softmax denominator: each exp group's two kt tiles are pair-summed
  in bf16 (SBUF->SBUF, split between DVE and the otherwise-idle gpsimd
  engine), then ones-MATRIX [128,128] stationary matmuls accumulate the
  pairs -- half the PE columns, and the PSUM result lands replicated
  across all 128 partitions -> no partition_broadcast; reciprocal runs
  full width and feeds the normalize multiply directly.
- E = exp(logits) in bf16; vT in bf16 via bf16 PE transpose (1 cyc/row).
- Software pipeline: attention+exp run THREE chunks ahead of the U/
  denominator matmuls (PE never waits on the Activation engine), Wa lags
  one more chunk.  All 8 denominator matmuls issue at the top of each
  iteration so the reciprocal->normalize DVE chain overlaps the U
  matmuls instead of gating the next iteration.
- Sample-0 conv phase is x-DMA-paced; chunks 0-2 attention groups and
  chunks 0-1's U/denominator accumulation interleave into it as filler.
  Sample-1 convs spread over sample-0's attention iterations 1-5.
- engine split: Act = exp + q-copies + steady-state vT copies; DVE =
  pools, reciprocal, normalize, y-adds, conv-phase vT copies (gpsimd
  cannot touch PSUM); x loads on the SP HWDGE queue, y stores on Act's.
- y is stored bf16 (host upcasts to f32): halves store traffic and the
  tail drain; adds ~1e-3 rounding, still 2.7x under the 2e-2 gate.
- PSUM banks: attn ring 4 + conv/Wa-out shared ring 2 + denom 1 + U 1.
"""

import sys

import numpy as np

if "/opt/trn_rl_repo" not in sys.path:
    sys.path.insert(0, "/opt/trn_rl_repo")

B, C, H, W = 16, 256, 64, 64
CA = C // 8          # 32  attn channels
CS = C // 2          # 128 value channels
HWF = H * W          # 4096 spatial positions
HWP = HWF // 4       # 1024 pooled positions
SPC = 2              # samples per core
NCORES = 8
CHUNK = 512          # qq columns per chunk
NCHUNK = HWF // CHUNK       # 8
KT = HWP // 128             # 8 kk tiles of 128
KG = 2                      # kk tiles per exp group
NG = KT // KG               # 4 groups

_built = {}
LABELS = {}


def _lab(inst, txt):
    try:
        LABELS[inst.ins.name] = txt
    except Exception:
        pass


def _build_program():
    from contextlib import ExitStack

    import concourse.bass as bass
    import concourse.tile as tile
    from concourse import bacc, mybir

    f32 = mybir.dt.float32
    f32r = mybir.dt.float32r
    bf16 = mybir.dt.bfloat16
    fp8 = mybir.dt.float8e4
    Exp = mybir.ActivationFunctionType.Exp
    DR = mybir.MatmulPerfMode.DoubleRow

    nc = bacc.Bacc(
        "TRN2", target_bir_lowering=False, debug=False, enable_asserts=False
    )

    x_d = nc.dram_tensor("x", [SPC, C, HWF], bf16, kind="ExternalInput").ap()
    x8_d = nc.dram_tensor("x8", [SPC, C, HWF], fp8, kind="ExternalInput").ap()
    wqk_d = nc.dram_tensor("wqkT", [C, 2 * CA], bf16, kind="ExternalInput").ap()
    wv8_d = nc.dram_tensor("wv8T", [C, CS], fp8, kind="ExternalInput").ap()
    wa_d = nc.dram_tensor("waTg", [CS, C], f32r, kind="ExternalInput").ap()
    idb_d = nc.dram_tensor("identb", [128, 128], bf16, kind="ExternalInput").ap()
    on_d = nc.dram_tensor("onesm", [128, 128], bf16, kind="ExternalInput").ap()
    y_d = nc.dram_tensor("y", [SPC, C, HWF], bf16, kind="ExternalOutput").ap()

    with tile.TileContext(nc) as tc, ExitStack() as ctx:
        consts = ctx.enter_context(tc.tile_pool(name="consts", bufs=1))
        xp = ctx.enter_context(tc.tile_pool(name="xp", bufs=1))
        x8p = ctx.enter_context(tc.tile_pool(name="x8p", bufs=1))
        qp = ctx.enter_context(tc.tile_pool(name="qp", bufs=1))
        kvp = ctx.enter_context(tc.tile_pool(name="kvp", bufs=1))
        ep = ctx.enter_context(tc.tile_pool(name="ep", bufs=14))
        e2p = ctx.enter_context(tc.tile_pool(name="e2p", bufs=4))
        sp = ctx.enter_context(tc.tile_pool(name="sp", bufs=3))
        yp = ctx.enter_context(tc.tile_pool(name="yp", bufs=4))
        psA = ctx.enter_context(tc.tile_pool(name="psA", bufs=2, space="PSUM"))
        psCv = ctx.enter_context(tc.tile_pool(name="psCv", bufs=2, space="PSUM"))
        psS = ctx.enter_context(tc.tile_pool(name="psS", bufs=1, space="PSUM"))
        psUu = ctx.enter_context(tc.tile_pool(name="psUu", bufs=1, space="PSUM"))

        # constants + x loads on the SP queue, ordered so conv(0,0) can
        # start as early as possible: conv weights, first 512-col slices
        # of x/x8, then everything else.
        xr = {}
        x8r = {}
        for s in range(SPC):
            xr[s] = xp.tile([128, 2, HWF], bf16, name=f"xr{s}", tag=f"xr{s}")
            x8r[s] = x8p.tile([128, 2, HWF], fp8, name=f"x8r{s}", tag=f"x8{s}")
        xv = {s: x_d[s].rearrange("(t p) m -> p t m", p=128) for s in range(SPC)}
        x8v = {s: x8_d[s].rearrange("(t p) m -> p t m", p=128) for s in range(SPC)}

        def xload(s, lo, hi, with8, eng=None):
            eng = eng or nc.sync
            cs = slice(lo, hi)
            eng.dma_start(xr[s][:, :, cs], xv[s][:, :, cs])
            if with8:
                eng.dma_start(x8r[s][:, :, cs], x8v[s][:, :, cs])

        cs0 = slice(0, 512)
        nc.sync.dma_start(xr[0][:, :, cs0], xv[0][:, :, cs0])
        wqk = consts.tile([128, 2, 2 * CA], bf16, name="wqk")
        nc.scalar.dma_start(wqk[:], wqk_d.rearrange("(t p) m -> p t m", p=128))
        nc.scalar.dma_start(x8r[0][:, :, cs0], x8v[0][:, :, cs0])
        wv8 = consts.tile([128, 2, CS], fp8, name="wv8")
        nc.scalar.dma_start(wv8[:], wv8_d.rearrange("(t p) m -> p t m", p=128))

        wa = consts.tile([128, 2, 128], f32r, name="wa")
        nc.sync.dma_start(wa[:], wa_d.rearrange("p (t m) -> p t m", t=2))
        identb = consts.tile([128, 128], bf16, name="identb")
        nc.sync.dma_start(identb[:], idb_d)
        onesm = consts.tile([128, 128], bf16, name="onesm")
        nc.sync.dma_start(onesm[:], on_d)
        # prime the Exp activation table so the 1.3us table load is off
        # the critical path
        prime = consts.tile([1, 2], f32, name="prime")
        nc.scalar.activation(prime[:], onesm[0:1, 0:2], Exp)

        xload(0, 512, 1024, True)
        xload(0, 1024, 2048, True)
        xload(0, 2048, 3072, True)
        xload(0, 3072, 4096, True)
        for h in range(4):
            xload(1, h * 1024, (h + 1) * 1024, h % 2 == 0)
            if h % 2 == 0:
                cs = slice(h * 1024 + 1024, h * 1024 + 2048)
                nc.sync.dma_start(x8r[1][:, :, cs], x8v[1][:, :, cs])

        # per-sample conv products
        qs = {}     # (s, ck) -> [32, 512] f32r q chunk
        kph = {}    # (s, kt) -> [32, 128] f32r pooled k tile
        vTh = {}    # (s, kt) -> [128, 128] bf16 v^T tile

        def conv_chunk(s, ck):
            """1x1 convs + pools + v transpose for chunk ck of sample s."""
            cs = slice(ck * CHUNK, (ck + 1) * CHUNK)
            # q/k conv, f32r, out [64, 512]
            pqk = psCv.tile([64, CHUNK], f32, name=f"pqk{s}{ck}", tag="cv")
            for t in range(2):
                _lab(nc.tensor.matmul(
                    pqk[:],
                    wqk[:, t, :],
                    xr[s][:, t, cs],
                    start=(t == 0),
                    stop=(t == 1),
                ), f"qkconv s{s}c{ck}t{t}")
            # v conv, fp8 DoubleRow: both 128-row tiles in one pass
            pv = psCv.tile([128, CHUNK], f32, name=f"pv{s}{ck}", tag="cv")
            _lab(nc.tensor.matmul(
                pv[:],
                wv8[:],
                x8r[s][:, :, cs],
                start=True,
                stop=True,
                perf_mode=DR,
            ), f"vconv s{s}c{ck}")
            # q chunk to SBUF on Act (keeps DVE off the critical chains)
            q = qp.tile([CA, CHUNK], f32r, name=f"q{s}{ck}", tag=f"q{s}{ck}")
            nc.scalar.copy(q[:], pqk[0:CA, :])
            qs[(s, ck)] = q
            # pooled k tile (DVE), cross-partition write 32:64 -> 0:32
            kt_ = kvp.tile([CA, 128], f32r, name=f"kp{s}{ck}", tag=f"kp{s}{ck}")
            nc.vector.tensor_reduce(
                kt_[:].rearrange("p (h2 w2) -> p h2 w2", h2=4),
                pqk[CA : 2 * CA, :].rearrange(
                    "p (h2 dh w2 dw) -> p h2 w2 dh dw", h2=4, dh=2, w2=32, dw=2
                ),
                axis=mybir.AxisListType.XY,
                op=mybir.AluOpType.max,
            )
            kph[(s, ck)] = kt_
            # pooled v tile (DVE) in bf16
            vph = kvp.tile([128, 128], bf16, name=f"vp{s}{ck}", tag=f"vp{s}{ck}")
            nc.vector.tensor_reduce(
                vph[:].rearrange("p (h2 w2) -> p h2 w2", h2=4),
                pv[:].rearrange(
                    "p (h2 dh w2 dw) -> p h2 w2 dh dw", h2=4, dh=2, w2=32, dw=2
                ),
                axis=mybir.AxisListType.XY,
                op=mybir.AluOpType.max,
            )
            vphs[(s, ck)] = vph

        vphs = {}

        def transpose_chunk(s, ck, dve_copy=False):
            # v^T via bf16 PE transpose, copy out on Act (or DVE near the
            # conv-phase tail where Act's queue is full of exps)
            ptr = psCv.tile([128, 128], bf16, name=f"pt{s}{ck}", tag="cv")
            _lab(nc.tensor.transpose(ptr[:], vphs[(s, ck)][:], identb[:]), f"transp s{s}c{ck}")
            vT = kvp.tile([128, 128], bf16, name=f"vT{s}{ck}", tag=f"vT{s}{ck}")
            if dve_copy:
                nc.vector.tensor_copy(vT[:], ptr[:])
            else:
                nc.scalar.copy(vT[:], ptr[:])
            vTh[(s, ck)] = vT

        # pipelined attention state
        st = {}

        def attn_groups(s, ck, glist):
            d = st[(s, ck)]
            for g in glist:
                pa = psA.tile([128, KG, CHUNK], f32, name=f"pa{s}{ck}{g}", tag="attn")
                for j in range(KG):
                    kt = g * KG + j
                    _lab(nc.tensor.matmul(
                        pa[:, j, :],
                        kph[(s, kt)][:],
                        qs[(s, ck)][:],
                        start=True,
                        stop=True,
                    ), f"attn s{s}c{ck}g{g}j{j}")
                eg = ep.tile([128, KG, CHUNK], bf16, name=f"E{s}{ck}{g}", tag="E")
                nc.scalar.activation(eg[:], pa[:], Exp)
                d["E"][g] = eg

        def u_mms(s, ck, gs):
            """U matmuls for exp groups gs of chunk ck."""
            d = st[(s, ck)]
            for g in gs:
                for j in range(KG):
                    kt = g * KG + j
                    _lab(nc.tensor.matmul(
                        d["psU"][:],
                        vTh[(s, kt)][:],
                        d["E"][g][:, j, :],
                        start=(kt == 0),
                        stop=(kt == KT - 1),
                    ), f"U s{s}c{ck}g{g}j{j}")

        def build_pairs(s, ck):
            """bf16 pair-sums of each exp group's two kt tiles, halving
            the denominator matmul count on the PE.  Pure SBUF->SBUF, so
            half run on the otherwise-idle gpsimd engine."""
            d = st[(s, ck)]
            pair = e2p.tile([128, NG, CHUNK], bf16, name=f"e2{s}{ck}", tag="e2")
            for g in range(NG):
                eng = nc.vector if g % 2 == 0 else nc.gpsimd
                eng.tensor_tensor(
                    pair[:, g, :],
                    d["E"][g][:, 0, :],
                    d["E"][g][:, 1, :],
                    op=mybir.AluOpType.add,
                )
            d["pair"] = pair

        def s_mms_paired(s, ck):
            d = st[(s, ck)]
            for g in range(NG):
                _lab(nc.tensor.matmul(
                    d["psS"][:],
                    onesm[:],
                    d["pair"][:, g, :],
                    start=(g == 0),
                    stop=(g == NG - 1),
                ), f"S2 s{s}c{ck}g{g}")

        def s_mms(s, ck, gs=tuple(range(NG))):
            """denominator matmuls of chunk ck (inputs ready: attn runs
            two chunks ahead), so the reciprocal can fire mid-iteration."""
            d = st[(s, ck)]
            for g in gs:
                for j in range(KG):
                    kt = g * KG + j
                    _lab(nc.tensor.matmul(
                        d["psS"][:],
                        onesm[:],
                        d["E"][g][:, j, :],
                        start=(kt == 0),
                        stop=(kt == KT - 1),
                    ), f"S s{s}c{ck}g{g}j{j}")

        def recip_chunk(s, ck):
            d = st[(s, ck)]
            rb = sp.tile([128, CHUNK], f32, name=f"rb{s}{ck}", tag="rb")
            nc.vector.reciprocal_approx_fast(rb[:], d["psS"][:])
            d["rb"] = rb

        def un_chunk(s, ck):
            d = st[(s, ck)]
            un = sp.tile([128, CHUNK], f32r, name=f"un{s}{ck}", tag="un")
            nc.vector.tensor_mul(un[:], d["psU"][:], d["rb"][:])
            d["un"] = un

        def wa_chunk(s, ck, split_store=False):
            d = st[(s, ck)]
            cs = slice(ck * CHUNK, (ck + 1) * CHUNK)
            yt = yp.tile([128, 2, CHUNK], bf16, name=f"yt{s}{ck}", tag="yt")
            yv = y_d[s].rearrange("(t p) m -> p t m", p=128)
            pos = []
            for mt in range(2):
                po = psCv.tile([128, CHUNK], f32, name=f"po{s}{ck}{mt}", tag="cv")
                _lab(nc.tensor.matmul(
                    po[:], wa[:, mt, :], d["un"][:], start=True, stop=True
                ), f"wa s{s}c{ck}mt{mt}")
                pos.append(po)
            # the tail chunks' stores ride the idle SP queue so they do
            # not serialize behind Act's earlier stores in the drain
            dq = nc.sync if split_store else nc.scalar
            for mt in range(2):
                nc.vector.tensor_tensor(
                    yt[:, mt, :],
                    pos[mt][:],
                    xr[s][:, mt, cs],
                    op=mybir.AluOpType.add,
                )
                if split_store:
                    dq.dma_start(yv[:, mt, cs], yt[:, mt, :])
            if not split_store:
                dq.dma_start(yv[:, :, cs], yt[:])

        def new_chunk_state(s, ck):
            st[(s, ck)] = {
                "E": [None] * NG,
                "psS": psS.tile([128, CHUNK], f32, name=f"ps{s}{ck}", tag="s"),
                "psU": psUu.tile([128, CHUNK], f32, name=f"pu{s}{ck}", tag="u"),
            }

        # ---- conv phase for sample 0.  x arrives over ~13us, so the
        # conv chain is DMA-paced; attention groups of chunks 0 and 1 are
        # issued as soon as their k tiles exist to keep the PE fed.
        # Transposes run two chunks behind their v-pool.
        new_chunk_state(0, 0)
        new_chunk_state(0, 1)
        new_chunk_state(0, 2)
        conv_filler = {
            3: [(0, 0)], 4: [(0, 1)], 5: [(1, 0), (0, 2)],
            6: [(1, 1), (1, 2), (2, 0)], 7: [(0, 3), (1, 3), (2, 1)],
        }
        for ck in range(NCHUNK):
            conv_chunk(0, ck)
            if ck >= 2:
                transpose_chunk(0, ck - 2, dve_copy=True)
            for c, g in conv_filler.get(ck, []):
                attn_groups(0, c, [g])
            # chunk (0,0)'s U/denominator accumulation starts as soon as
            # its exp groups and vT tiles exist
            if ck >= 5:
                s_mms(0, 0, [ck - 5])
                u_mms(0, 0, [ck - 5])
        transpose_chunk(0, NCHUNK - 2, dve_copy=True)
        transpose_chunk(0, NCHUNK - 1, dve_copy=True)
        attn_groups(0, 2, [2, 3])
        s_mms(0, 0, [3])
        u_mms(0, 0, [3])
        recip_chunk(0, 0)
        un_chunk(0, 0)
        # chunk (0,1)'s accumulation also starts here: its exps and vT
        # tiles all exist by the end of the conv phase
        s_mms(0, 1, [0, 1])
        u_mms(0, 1, [0, 1])

        # ---- pipelined attention over 16 virtual chunks: attention runs
        # TWO chunks ahead of the U/denominator matmuls (so exp latency
        # never gates the PE), Wa lags U by one chunk.
        VC = [(s, ck) for s in range(SPC) for ck in range(NCHUNK)]

        for i, (s, ck) in enumerate(VC):
            ahead = VC[i + 3] if i + 3 < len(VC) else None
            prev = VC[i - 1] if i >= 1 else None
            if ahead is not None and i >= 0:
                if ahead not in st:
                    new_chunk_state(*ahead)
                    attn_groups(*ahead, [0, 1])
            last = i == len(VC) - 1
            if i == 1:
                s_mms(s, ck, [2, 3])
                recip_chunk(s, ck)
            elif i > 1:
                s_mms_paired(s, ck)
                recip_chunk(s, ck)
            if i + 1 < len(VC) and i >= 1:
                build_pairs(*VC[i + 1])
            if ahead is not None and ahead[0] * NCHUNK + ahead[1] > 2:
                attn_groups(*ahead, [2])
            if i > 1:
                u_mms(s, ck, [0, 1])
            if i > 0 and last:
                u_mms(s, ck, [2, 3])
                un_chunk(s, ck)
            if prev is not None:
                wa_chunk(*prev, split_store=last)
            # sample-1 convs spread over iters 1-5 (pairs then singles),
            # issued before the ahead-attn g3 so iter 5's chunk (1,0)
            # lookahead has every k tile
            s1c = {1: [0, 1], 2: [2, 3], 3: [4, 5], 4: [6], 5: [7]}.get(i, [])
            for c1 in s1c:
                conv_chunk(1, c1)
                if c1 >= 1:
                    transpose_chunk(1, c1 - 1)
            if ahead is not None and ahead[0] * NCHUNK + ahead[1] > 2:
                attn_groups(*ahead, [3])
            if i > 0 and not last:
                u_mms(s, ck, [2, 3])
                un_chunk(s, ck)
            if i == 6:
                transpose_chunk(1, NCHUNK - 1, dve_copy=True)

        # drain tail
        wa_chunk(*VC[-1], split_store=True)

    nc.compile()
    return nc


def _get_program():
    if "nc" not in _built:
        _built["nc"] = _build_program()
    return _built["nc"]


def _make_in_maps(x, Wq, Wk, Wv, Wa, gamma):
    import ml_dtypes

    fp8 = ml_dtypes.float8_e4m3
    xf = np.ascontiguousarray(np.asarray(x, dtype=np.float32).reshape(B, C, HWF))
    x8 = np.ascontiguousarray(xf.astype(fp8))
    x = np.ascontiguousarray(xf.astype(ml_dtypes.bfloat16))
    wqkT = np.ascontiguousarray(
        np.concatenate([np.asarray(Wq), np.asarray(Wk)], axis=0).T.astype(
            ml_dtypes.bfloat16
        )
    )
    wv8T = np.ascontiguousarray(np.asarray(Wv).T.astype(np.float32)).astype(fp8)
    waTg = np.ascontiguousarray(
        (float(np.asarray(gamma).reshape(-1)[0]) * np.asarray(Wa)).T.astype(np.float32)
    )
    identb = np.eye(128, dtype=np.float32).astype(ml_dtypes.bfloat16)
    onesm = np.ones((128, 128), dtype=np.float32).astype(ml_dtypes.bfloat16)
    return [
        {
            "x": np.ascontiguousarray(x[c * SPC : (c + 1) * SPC]),
            "x8": np.ascontiguousarray(x8[c * SPC : (c + 1) * SPC]),
            "wqkT": wqkT,
            "wv8T": wv8T,
            "waTg": waTg,
            "identb": identb,
            "onesm": onesm,
        }
        for c in range(NCORES)
    ]


def kernel(x, Wq, Wk, Wv, Wa, gamma):
    from concourse import bass_utils

    nc = _get_program()
    in_maps = _make_in_maps(x, Wq, Wk, Wv, Wa, gamma)
    res = bass_utils.run_bass_kernel_spmd(
        nc, in_maps, core_ids=list(range(NCORES))
    )
    out = np.concatenate(
        [np.asarray(res.results[c]["y"], dtype=np.float32) for c in range(NCORES)],
        axis=0,
    )
    return out.reshape(B, C, H, W)


# revision 50
# speedup vs baseline: 1.0027x; 1.0009x over previous
"""Trainium2 Bass kernel for nn_Attention_5093831213465.

Reference computation (per sample, x_b: [256, 4096]):
  q = Wq @ x_b                       [32, 4096]
  k = maxpool2(Wk @ x_b)             [32, 1024]
  v = maxpool2(Wv @ x_b)             [128, 1024]
  attn = softmax_over_k(k^T @ q)     [1024, 4096]
  out  = Wa @ (v @ attn)             [256, 4096]
  y    = gamma * out + x_b

Sharding: data-parallel over batch, 2 samples per core on 8 cores.

Design (tuned against the TimelineSim cost model: a matmul costs
output_free_cols * pe_cycle * cycles_per_row regardless of contraction
or partition dims; fp8 DoubleRow halves cycles/row; PE drops out of its
fast p-state after any idle gap, so the PE queue must never drain):
- x ships three ways: bf16 (q/k conv fmap + residual), fp8e4 (v conv),
  never f32 -- halves the x DMA traffic that paces the warmup.
- q/k conv: stacked [Wq;Wk] bf16 weight, out [64, 512] per chunk.
- v conv: fp8e4 DoubleRow, both 128-row contraction tiles in one pass
  (0.5 cyc/row).  fp8 noise only touches the value path, never the
  softmax logits (q/k stay bf16xbf16 -> fp32 PSUM).
- softmax denominator: each exp group's two kt tiles are pair-summed
  in bf16 (SBUF->SBUF, split between DVE and the otherwise-idle gpsimd
  engine), then ones-MATRIX [128,128] stationary matmuls accumulate the
  pairs -- half the PE columns, and the PSUM result lands replicated
  across all 128 partitions -> no partition_broadcast; reciprocal runs
  full width and feeds the normalize multiply directly.
- E = exp(logits) in bf16; vT in bf16 via bf16 PE transpose (1 cyc/row).
- Software pipeline: attention+exp run THREE chunks ahead of the U/
  denominator matmuls (PE never waits on the Activation engine), Wa lags
  one more chunk.  All 8 denominator matmuls issue at the top of each
  iteration so the reciprocal->normalize DVE chain overlaps the U
  matmuls instead of gating the next iteration.
- Sample-0 conv phase is x-DMA-paced; chunks 0-2 attention groups and
  chunks 0-1's U/denominator accumulation interleave into it as filler.
  Sample-1 convs spread over sample-0's attention iterations 1-5.
- engine split: Act = exp + q-copies + steady-state vT copies; DVE =
  pools, reciprocal, normalize, y-adds, conv-phase vT copies (gpsimd
  cannot touch PSUM); x loads on the SP HWDGE queue, y stores on Act's.
- y is stored bf16 (host upcasts to f32): halves store traffic and the
  tail drain; adds ~1e-3 rounding, still 2.7x under the 2e-2 gate.
- PSUM banks: attn ring 4 + conv/Wa-out shared ring 2 + denom 1 + U 1.
"""

import sys

import numpy as np

if "/opt/trn_rl_repo" not in sys.path:
    sys.path.insert(0, "/opt/trn_rl_repo")

B, C, H, W = 16, 256, 64, 64
CA = C // 8          # 32  attn channels
CS = C // 2          # 128 value channels
HWF = H * W          # 4096 spatial positions
HWP = HWF // 4       # 1024 pooled positions
SPC = 2              # samples per core
NCORES = 8
CHUNK = 512          # qq columns per chunk
NCHUNK = HWF // CHUNK       # 8
KT = HWP // 128             # 8 kk tiles of 128
KG = 2                      # kk tiles per exp group
NG = KT // KG               # 4 groups

_built = {}
LABELS = {}


def _lab(inst, txt):
    try:
        LABELS[inst.ins.name] = txt
    except Exception:
        pass


def _build_program():
    from contextlib import ExitStack

    import concourse.bass as bass
    import concourse.tile as tile
    from concourse import bacc, mybir

    f32 = mybir.dt.float32
    f32r = mybir.dt.float32r
    bf16 = mybir.dt.bfloat16
    fp8 = mybir.dt.float8e4
    Exp = mybir.ActivationFunctionType.Exp
    DR = mybir.MatmulPerfMode.DoubleRow

    nc = bacc.Bacc(
        "TRN2", target_bir_lowering=False, debug=False, enable_asserts=False
    )

    x_d = nc.dram_tensor("x", [SPC, C, HWF], bf16, kind="ExternalInput").ap()
    x8_d = nc.dram_tensor("x8", [SPC, C, HWF], fp8, kind="ExternalInput").ap()
    wqk_d = nc.dram_tensor("wqkT", [C, 2 * CA], bf16, kind="ExternalInput").ap()
    wv8_d = nc.dram_tensor("wv8T", [C, CS], fp8, kind="ExternalInput").ap()
    wa_d = nc.dram_tensor("waTg", [CS, C], f32r, kind="ExternalInput").ap()
    idb_d = nc.dram_tensor("identb", [128, 128], bf16, kind="ExternalInput").ap()
    on_d = nc.dram_tensor("onesm", [128, 128], bf16, kind="ExternalInput").ap()
    y_d = nc.dram_tensor("y", [SPC, C, HWF], bf16, kind="ExternalOutput").ap()

    with tile.TileContext(nc) as tc, ExitStack() as ctx:
        consts = ctx.enter_context(tc.tile_pool(name="consts", bufs=1))
        xp = ctx.enter_context(tc.tile_pool(name="xp", bufs=1))
        x8p = ctx.enter_context(tc.tile_pool(name="x8p", bufs=1))
        qp = ctx.enter_context(tc.tile_pool(name="qp", bufs=1))
        kvp = ctx.enter_context(tc.tile_pool(name="kvp", bufs=1))
        ep = ctx.enter_context(tc.tile_pool(name="ep", bufs=14))
        e2p = ctx.enter_context(tc.tile_pool(name="e2p", bufs=4))
        sp = ctx.enter_context(tc.tile_pool(name="sp", bufs=3))
        yp = ctx.enter_context(tc.tile_pool(name="yp", bufs=4))
        psA = ctx.enter_context(tc.tile_pool(name="psA", bufs=2, space="PSUM"))
        psCv = ctx.enter_context(tc.tile_pool(name="psCv", bufs=2, space="PSUM"))
        psS = ctx.enter_context(tc.tile_pool(name="psS", bufs=1, space="PSUM"))
        psUu = ctx.enter_context(tc.tile_pool(name="psUu", bufs=1, space="PSUM"))

        # constants + x loads on the SP queue, ordered so conv(0,0) can
        # start as early as possible: conv weights, first 512-col slices
        # of x/x8, then everything else.
        xr = {}
        x8r = {}
        for s in range(SPC):
            xr[s] = xp.tile([128, 2, HWF], bf16, name=f"xr{s}", tag=f"xr{s}")
            x8r[s] = x8p.tile([128, 2, HWF], fp8, name=f"x8r{s}", tag=f"x8{s}")
        xv = {s: x_d[s].rearrange("(t p) m -> p t m", p=128) for s in range(SPC)}
        x8v = {s: x8_d[s].rearrange("(t p) m -> p t m", p=128) for s in range(SPC)}

        def xload(s, lo, hi, with8, eng=None):
            eng = eng or nc.sync
            cs = slice(lo, hi)
            eng.dma_start(xr[s][:, :, cs], xv[s][:, :, cs])
            if with8:
                eng.dma_start(x8r[s][:, :, cs], x8v[s][:, :, cs])

        cs0 = slice(0, 512)
        nc.sync.dma_start(xr[0][:, :, cs0], xv[0][:, :, cs0])
        wqk = consts.tile([128, 2, 2 * CA], bf16, name="wqk")
        nc.scalar.dma_start(wqk[:], wqk_d.rearrange("(t p) m -> p t m", p=128))
        nc.scalar.dma_start(x8r[0][:, :, cs0], x8v[0][:, :, cs0])
        wv8 = consts.tile([128, 2, CS], fp8, name="wv8")
        nc.scalar.dma_start(wv8[:], wv8_d.rearrange("(t p) m -> p t m", p=128))

        wa = consts.tile([128, 2, 128], f32r, name="wa")
        nc.sync.dma_start(wa[:], wa_d.rearrange("p (t m) -> p t m", t=2))
        identb = consts.tile([128, 128], bf16, name="identb")
        nc.sync.dma_start(identb[:], idb_d)
        onesm = consts.tile([128, 128], bf16, name="onesm")
        nc.sync.dma_start(onesm[:], on_d)
        # prime the Exp activation table so the 1.3us table load is off
        # the critical path
        prime = consts.tile([1, 2], f32, name="prime")
        nc.scalar.activation(prime[:], onesm[0:1, 0:2], Exp)

        xload(0, 512, 1024, True)
        for hh in range(2, 8):
            xload(0, hh * 512, (hh + 1) * 512, True)
        for h in range(4):
            xload(1, h * 1024, (h + 1) * 1024, h % 2 == 0)
            if h % 2 == 0:
                cs = slice(h * 1024 + 1024, h * 1024 + 2048)
                nc.sync.dma_start(x8r[1][:, :, cs], x8v[1][:, :, cs])

        # per-sample conv products
        qs = {}     # (s, ck) -> [32, 512] f32r q chunk
        kph = {}    # (s, kt) -> [32, 128] f32r pooled k tile
        vTh = {}    # (s, kt) -> [128, 128] bf16 v^T tile

        def conv_chunk(s, ck):
            """1x1 convs + pools + v transpose for chunk ck of sample s."""
            cs = slice(ck * CHUNK, (ck + 1) * CHUNK)
            # q/k conv, f32r, out [64, 512]
            pqk = psCv.tile([64, CHUNK], f32, name=f"pqk{s}{ck}", tag="cv")
            for t in range(2):
                _lab(nc.tensor.matmul(
                    pqk[:],
                    wqk[:, t, :],
                    xr[s][:, t, cs],
                    start=(t == 0),
                    stop=(t == 1),
                ), f"qkconv s{s}c{ck}t{t}")
            # v conv, fp8 DoubleRow: both 128-row tiles in one pass
            pv = psCv.tile([128, CHUNK], f32, name=f"pv{s}{ck}", tag="cv")
            _lab(nc.tensor.matmul(
                pv[:],
                wv8[:],
                x8r[s][:, :, cs],
                start=True,
                stop=True,
                perf_mode=DR,
            ), f"vconv s{s}c{ck}")
            # q chunk to SBUF on Act (keeps DVE off the critical chains)
            q = qp.tile([CA, CHUNK], f32r, name=f"q{s}{ck}", tag=f"q{s}{ck}")
            nc.scalar.copy(q[:], pqk[0:CA, :])
            qs[(s, ck)] = q
            # pooled k tile (DVE), cross-partition write 32:64 -> 0:32
            kt_ = kvp.tile([CA, 128], f32r, name=f"kp{s}{ck}", tag=f"kp{s}{ck}")
            nc.vector.tensor_reduce(
                kt_[:].rearrange("p (h2 w2) -> p h2 w2", h2=4),
                pqk[CA : 2 * CA, :].rearrange(
                    "p (h2 dh w2 dw) -> p h2 w2 dh dw", h2=4, dh=2, w2=32, dw=2
                ),
                axis=mybir.AxisListType.XY,
                op=mybir.AluOpType.max,
            )
            kph[(s, ck)] = kt_
            # pooled v tile (DVE) in bf16
            vph = kvp.tile([128, 128], bf16, name=f"vp{s}{ck}", tag=f"vp{s}{ck}")
            nc.vector.tensor_reduce(
                vph[:].rearrange("p (h2 w2) -> p h2 w2", h2=4),
                pv[:].rearrange(
                    "p (h2 dh w2 dw) -> p h2 w2 dh dw", h2=4, dh=2, w2=32, dw=2
                ),
                axis=mybir.AxisListType.XY,
                op=mybir.AluOpType.max,
            )
            vphs[(s, ck)] = vph

        vphs = {}

        def transpose_chunk(s, ck, dve_copy=False):
            # v^T via bf16 PE transpose, copy out on Act (or DVE near the
            # conv-phase tail where Act's queue is full of exps)
            ptr = psCv.tile([128, 128], bf16, name=f"pt{s}{ck}", tag="cv")
            _lab(nc.tensor.transpose(ptr[:], vphs[(s, ck)][:], identb[:]), f"transp s{s}c{ck}")
            vT = kvp.tile([128, 128], bf16, name=f"vT{s}{ck}", tag=f"vT{s}{ck}")
            if dve_copy:
                nc.vector.tensor_copy(vT[:], ptr[:])
            else:
                nc.scalar.copy(vT[:], ptr[:])
            vTh[(s, ck)] = vT

        # pipelined attention state
        st = {}

        def attn_groups(s, ck, glist):
            d = st[(s, ck)]
            for g in glist:
                pa = psA.tile([128, KG, CHUNK], f32, name=f"pa{s}{ck}{g}", tag="attn")
                for j in range(KG):
                    kt = g * KG + j
                    _lab(nc.tensor.matmul(
                        pa[:, j, :],
                        kph[(s, kt)][:],
                        qs[(s, ck)][:],
                        start=True,
                        stop=True,
                    ), f"attn s{s}c{ck}g{g}j{j}")
                eg = ep.tile([128, KG, CHUNK], bf16, name=f"E{s}{ck}{g}", tag="E")
                nc.scalar.activation(eg[:], pa[:], Exp)
                d["E"][g] = eg

        def u_mms(s, ck, gs):
            """U matmuls for exp groups gs of chunk ck."""
            d = st[(s, ck)]
            for g in gs:
                for j in range(KG):
                    kt = g * KG + j
                    _lab(nc.tensor.matmul(
                        d["psU"][:],
                        vTh[(s, kt)][:],
                        d["E"][g][:, j, :],
                        start=(kt == 0),
                        stop=(kt == KT - 1),
                    ), f"U s{s}c{ck}g{g}j{j}")

        def build_pairs(s, ck):
            """bf16 pair-sums of each exp group's two kt tiles, halving
            the denominator matmul count on the PE.  Pure SBUF->SBUF, so
            half run on the otherwise-idle gpsimd engine."""
            d = st[(s, ck)]
            pair = e2p.tile([128, NG, CHUNK], bf16, name=f"e2{s}{ck}", tag="e2")
            for g in range(NG):
                eng = nc.vector if g % 2 == 0 else nc.gpsimd
                eng.tensor_tensor(
                    pair[:, g, :],
                    d["E"][g][:, 0, :],
                    d["E"][g][:, 1, :],
                    op=mybir.AluOpType.add,
                )
            d["pair"] = pair

        def s_mms_paired(s, ck):
            d = st[(s, ck)]
            for g in range(NG):
                _lab(nc.tensor.matmul(
                    d["psS"][:],
                    onesm[:],
                    d["pair"][:, g, :],
                    start=(g == 0),
                    stop=(g == NG - 1),
                ), f"S2 s{s}c{ck}g{g}")

        def s_mms(s, ck, gs=tuple(range(NG))):
            """denominator matmuls of chunk ck (inputs ready: attn runs
            two chunks ahead), so the reciprocal can fire mid-iteration."""
            d = st[(s, ck)]
            for g in gs:
                for j in range(KG):
                    kt = g * KG + j
                    _lab(nc.tensor.matmul(
                        d["psS"][:],
                        onesm[:],
                        d["E"][g][:, j, :],
                        start=(kt == 0),
                        stop=(kt == KT - 1),
                    ), f"S s{s}c{ck}g{g}j{j}")

        def recip_chunk(s, ck):
            d = st[(s, ck)]
            rb = sp.tile([128, CHUNK], f32, name=f"rb{s}{ck}", tag="rb")
            nc.vector.reciprocal_approx_fast(rb[:], d["psS"][:])
            d["rb"] = rb

        def un_chunk(s, ck):
            d = st[(s, ck)]
            un = sp.tile([128, CHUNK], f32r, name=f"un{s}{ck}", tag="un")
            nc.vector.tensor_mul(un[:], d["psU"][:], d["rb"][:])
            d["un"] = un

        def wa_chunk(s, ck, split_store=False):
            d = st[(s, ck)]
            cs = slice(ck * CHUNK, (ck + 1) * CHUNK)
            yt = yp.tile([128, 2, CHUNK], bf16, name=f"yt{s}{ck}", tag="yt")
            yv = y_d[s].rearrange("(t p) m -> p t m", p=128)
            pos = []
            for mt in range(2):
                po = psCv.tile([128, CHUNK], f32, name=f"po{s}{ck}{mt}", tag="cv")
                _lab(nc.tensor.matmul(
                    po[:], wa[:, mt, :], d["un"][:], start=True, stop=True
                ), f"wa s{s}c{ck}mt{mt}")
                pos.append(po)
            # the tail chunks' stores ride the idle SP queue so they do
            # not serialize behind Act's earlier stores in the drain
            dq = nc.sync if split_store else nc.scalar
            for mt in range(2):
                nc.vector.tensor_tensor(
                    yt[:, mt, :],
                    pos[mt][:],
                    xr[s][:, mt, cs],
                    op=mybir.AluOpType.add,
                )
                if split_store:
                    dq.dma_start(yv[:, mt, cs], yt[:, mt, :])
            if not split_store:
                dq.dma_start(yv[:, :, cs], yt[:])

        def new_chunk_state(s, ck):
            st[(s, ck)] = {
                "E": [None] * NG,
                "psS": psS.tile([128, CHUNK], f32, name=f"ps{s}{ck}", tag="s"),
                "psU": psUu.tile([128, CHUNK], f32, name=f"pu{s}{ck}", tag="u"),
            }

        # ---- conv phase for sample 0.  x arrives over ~13us, so the
        # conv chain is DMA-paced; attention groups of chunks 0 and 1 are
        # issued as soon as their k tiles exist to keep the PE fed.
        # Transposes run two chunks behind their v-pool.
        new_chunk_state(0, 0)
        new_chunk_state(0, 1)
        new_chunk_state(0, 2)
        conv_filler = {
            3: [(0, 0)], 4: [(0, 1)], 5: [(1, 0), (0, 2)],
            6: [(1, 1), (1, 2), (2, 0)], 7: [(0, 3), (1, 3), (2, 1)],
        }
        for ck in range(NCHUNK):
            conv_chunk(0, ck)
            if ck >= 2:
                transpose_chunk(0, ck - 2, dve_copy=True)
            for c, g in conv_filler.get(ck, []):
                attn_groups(0, c, [g])
            # chunk (0,0)'s U/denominator accumulation starts as soon as
            # its exp groups and vT tiles exist
            if ck >= 5:
                s_mms(0, 0, [ck - 5])
                u_mms(0, 0, [ck - 5])
        transpose_chunk(0, NCHUNK - 2, dve_copy=True)
        transpose_chunk(0, NCHUNK - 1, dve_copy=True)
        attn_groups(0, 2, [2, 3])
        s_mms(0, 0, [3])
        u_mms(0, 0, [3])
        recip_chunk(0, 0)
        un_chunk(0, 0)
        # chunk (0,1)'s accumulation also starts here: its exps and vT
        # tiles all exist by the end of the conv phase
        s_mms(0, 1, [0, 1])
        u_mms(0, 1, [0, 1])

        # ---- pipelined attention over 16 virtual chunks: attention runs
        # TWO chunks ahead of the U/denominator matmuls (so exp latency
        # never gates the PE), Wa lags U by one chunk.
        VC = [(s, ck) for s in range(SPC) for ck in range(NCHUNK)]

        for i, (s, ck) in enumerate(VC):
            ahead = VC[i + 3] if i + 3 < len(VC) else None
            prev = VC[i - 1] if i >= 1 else None
            if ahead is not None and i >= 0:
                if ahead not in st:
                    new_chunk_state(*ahead)
                    attn_groups(*ahead, [0, 1])
            last = i == len(VC) - 1
            if i == 1:
                s_mms(s, ck, [2, 3])
                recip_chunk(s, ck)
            elif i > 1:
                s_mms_paired(s, ck)
                recip_chunk(s, ck)
            if i + 1 < len(VC) and i >= 1:
                build_pairs(*VC[i + 1])
            if ahead is not None and ahead[0] * NCHUNK + ahead[1] > 2:
                attn_groups(*ahead, [2])
            if i > 1:
                u_mms(s, ck, [0, 1])
            if i > 0 and last:
                u_mms(s, ck, [2, 3])
                un_chunk(s, ck)
            if prev is not None:
                wa_chunk(*prev, split_store=last)
            # sample-1 convs spread over iters 1-5 (pairs then singles),
            # issued before the ahead-attn g3 so iter 5's chunk (1,0)
            # lookahead has every k tile
            s1c = {1: [0, 1], 2: [2, 3], 3: [4, 5], 4: [6], 5: [7]}.get(i, [])
            for c1 in s1c:
                conv_chunk(1, c1)
                if c1 >= 1:
                    transpose_chunk(1, c1 - 1)
            if ahead is not None and ahead[0] * NCHUNK + ahead[1] > 2:
                attn_groups(*ahead, [3])
            if i > 0 and not last:
                u_mms(s, ck, [2, 3])
                un_chunk(s, ck)
            if i == 6:
                transpose_chunk(1, NCHUNK - 1, dve_copy=True)

        # drain tail
        wa_chunk(*VC[-1], split_store=True)

    nc.compile()
    return nc


def _get_program():
    if "nc" not in _built:
        _built["nc"] = _build_program()
    return _built["nc"]


def _make_in_maps(x, Wq, Wk, Wv, Wa, gamma):
    import ml_dtypes

    fp8 = ml_dtypes.float8_e4m3
    xf = np.ascontiguousarray(np.asarray(x, dtype=np.float32).reshape(B, C, HWF))
    x8 = np.ascontiguousarray(xf.astype(fp8))
    x = np.ascontiguousarray(xf.astype(ml_dtypes.bfloat16))
    wqkT = np.ascontiguousarray(
        np.concatenate([np.asarray(Wq), np.asarray(Wk)], axis=0).T.astype(
            ml_dtypes.bfloat16
        )
    )
    wv8T = np.ascontiguousarray(np.asarray(Wv).T.astype(np.float32)).astype(fp8)
    waTg = np.ascontiguousarray(
        (float(np.asarray(gamma).reshape(-1)[0]) * np.asarray(Wa)).T.astype(np.float32)
    )
    identb = np.eye(128, dtype=np.float32).astype(ml_dtypes.bfloat16)
    onesm = np.ones((128, 128), dtype=np.float32).astype(ml_dtypes.bfloat16)
    return [
        {
            "x": np.ascontiguousarray(x[c * SPC : (c + 1) * SPC]),
            "x8": np.ascontiguousarray(x8[c * SPC : (c + 1) * SPC]),
            "wqkT": wqkT,
            "wv8T": wv8T,
            "waTg": waTg,
            "identb": identb,
            "onesm": onesm,
        }
        for c in range(NCORES)
    ]


def kernel(x, Wq, Wk, Wv, Wa, gamma):
    from concourse import bass_utils

    nc = _get_program()
    in_maps = _make_in_maps(x, Wq, Wk, Wv, Wa, gamma)
    res = bass_utils.run_bass_kernel_spmd(
        nc, in_maps, core_ids=list(range(NCORES))
    )
    out = np.concatenate(
        [np.asarray(res.results[c]["y"], dtype=np.float32) for c in range(NCORES)],
        axis=0,
    )
    return out.reshape(B, C, H, W)
